# revision 4
# baseline (speedup 1.0000x reference)
"""Multi-head causal self-attention (B=4, S=2048, D=1024, H=16) on 8 trn2 cores.

Sharding: core c = (batch b = c//2, head-group g = c%2 of 8 heads).
Each core computes Q/K/V projections for its 8 heads over its batch's 2048
tokens, causal attention, and a PARTIAL output projection over its 512
feature dims. The host adds the two partial [2048, 1024] outputs per batch.
No on-device collectives.

On-core dataflow (all f32):
  QT[e,t], KT[e,t] = W @ x^T          (e on partitions -> head-dim-major)
  V'[t, h, 0:64] = x @ Wv^T, col 64 = 1.0   (ones column makes PV matmul
                                             also produce the softmax sum Z)
  St[s,q] = K^T-slices .T @ Q^T-slices      (scores transposed; K=64 row-
                                             packed pairs of heads)
  E = exp(St/8) via ACT from PSUM; causal mask via gpsimd.affine_select
  numerT[dv,q], Z[q] = V'^T.T @ E            (accumulated over s-tiles)
  attnT = numerT * (1/Z broadcast)
  out[t,e] = attnT-slices .T @ Wo^T-slices   (partial over this core's dims)
"""

import numpy as np

import concourse.bass as bass
import concourse.mybir as mybir
import concourse.tile as tile
from concourse import bacc
from concourse.bass_utils import run_bass_kernel_spmd

F32 = mybir.dt.float32
AF = mybir.ActivationFunctionType

B = 4
S = 2048
D_MODEL = 1024
E = 512          # feature dims per core (8 heads x 64)
HEADS = 8        # heads per core
DK = 64
NQ = 4           # 512-token q-slices
ND = 8           # 128-dim d_model tiles
NT = 16          # 128-token tiles
SCALE = 0.125    # 1/sqrt(dk)


def _emit(tc, xT, wqT, wkT, wvT, woT, out):
    nc = tc.nc
    with tc.tile_pool(name="singles", bufs=1) as singles:
        qt = [singles.tile([128, S], F32, name=f"qt{i}") for i in range(4)]
        kt = [singles.tile([128, S], F32, name=f"kt{i}") for i in range(4)]
        vt = singles.tile([128, NT, HEADS, DK + 1], F32, name="vt")
        wot = singles.tile([128, 4, D_MODEL], F32, name="wot")

        nc.sync.dma_start(out=wot, in_=woT.rearrange("(n p) m -> p n m", p=128))
        nc.vector.memset(vt[:, :, :, DK:DK + 1], 1.0)

        # ---------------- Phase 1: Q/K/V projections ----------------
        with (
            tc.tile_pool(name="wpool", bufs=1) as wpool,
            tc.tile_pool(name="xtc", bufs=9) as xtp,
            tc.tile_pool(name="qkps", bufs=4, space="PSUM") as qkps,
        ):
            wq = wpool.tile([128, ND, E], F32, name="wq")
            wk = wpool.tile([128, ND, E], F32, name="wk")
            wv = wpool.tile([128, ND, E], F32, name="wv")
            nc.sync.dma_start(out=wq, in_=wqT.rearrange("(n p) m -> p n m", p=128))
            nc.sync.dma_start(out=wk, in_=wkT.rearrange("(n p) m -> p n m", p=128))
            nc.sync.dma_start(out=wv, in_=wvT.rearrange("(n p) m -> p n m", p=128))
            for j in range(NQ):
                tsl = slice(j * 512, (j + 1) * 512)
                chunks = []
                for d in range(ND):
                    ck = xtp.tile([128, 512], F32, tag="xc", name=f"xc{j}_{d}")
                    nc.sync.dma_start(out=ck, in_=xT[d * 128:(d + 1) * 128, tsl])
                    chunks.append(ck)
                for w, dst in ((wq, qt), (wk, kt)):
                    for e in range(4):
                        esl = slice(e * 128, (e + 1) * 128)
                        ps = qkps.tile([128, 512], F32, tag="qk", name=f"ps{j}_{e}")
                        for d in range(ND):
                            nc.tensor.matmul(ps, w[:, d, esl], chunks[d],
                                             start=(d == 0), stop=(d == ND - 1))
                        nc.vector.tensor_copy(dst[e][:, tsl], ps)
                for u in range(4):
                    i = 4 * j + u
                    usl = slice(u * 128, (u + 1) * 128)
                    ps = qkps.tile([128, 512], F32, tag="qk", name=f"psv{j}_{u}")
                    for d in range(ND):
                        nc.tensor.matmul(ps, chunks[d][:, usl], wv[:, d, :],
                                         start=(d == 0), stop=(d == ND - 1))
                    nc.vector.tensor_copy(
                        vt[:, i, :, 0:DK],
                        ps.rearrange("p (h k) -> p h k", h=HEADS))

        # ------------- Phase 2+3: attention + output projection -------------
        with (
            tc.tile_pool(name="stps", bufs=3, space="PSUM") as stps,
            tc.tile_pool(name="smps", bufs=2, space="PSUM") as smps,
            tc.tile_pool(name="expp", bufs=6) as expp,
            tc.tile_pool(name="attp", bufs=8) as attp,
            tc.tile_pool(name="nrmp", bufs=2) as nrmp,
            tc.tile_pool(name="outp", bufs=2) as outp,
        ):
            for j in range(NQ):
                qsl = slice(j * 512, (j + 1) * 512)
                att_j = [attp.tile([128, 512], F32, tag="at", name=f"at{j}_{f}")
                         for f in range(4)]
                for hp in range(4):
                    hA, hB = 2 * hp, 2 * hp + 1
                    pvA = smps.tile([DK + 1, 512], F32, tag="sp", name=f"pvA{j}{hp}")
                    pvB = smps.tile([DK + 1, 512], F32, tag="sp", name=f"pvB{j}{hp}")
                    nst = 4 * (j + 1)
                    for g in range(nst // 2):
                        stA = stps.tile([128, 2, 512], F32, tag="st", name=f"stA{j}{hp}{g}")
                        stB = stps.tile([128, 2, 512], F32, tag="st", name=f"stB{j}{hp}{g}")
                        for u in range(2):
                            i = 2 * g + u
                            ssl = slice(i * 128, (i + 1) * 128)
                            nc.tensor.matmul(stA[:, u, :], kt[hp][0:64, ssl],
                                             qt[hp][0:64, qsl], start=True, stop=True)
                            nc.tensor.matmul(stB[:, u, :], kt[hp][64:128, ssl],
                                             qt[hp][64:128, qsl], start=True, stop=True)
                        exA = expp.tile([128, 2, 512], F32, tag="ex", name=f"exA{j}{hp}{g}")
                        exB = expp.tile([128, 2, 512], F32, tag="ex", name=f"exB{j}{hp}{g}")
                        nc.scalar.activation(exA, stA, AF.Exp, scale=SCALE)
                        nc.scalar.activation(exB, stB, AF.Exp, scale=SCALE)
                        for u in range(2):
                            i = 2 * g + u
                            r = i - 4 * j
                            if r >= 0:  # diagonal tile: zero where s > q
                                for ex in (exA, exB):
                                    nc.gpsimd.affine_select(
                                        out=ex[:, u, :], in_=ex[:, u, :],
                                        pattern=[[1, 512]],
                                        compare_op=mybir.AluOpType.is_ge,
                                        fill=0.0, base=-128 * r,
                                        channel_multiplier=-1)
                            nc.tensor.matmul(pvA, vt[:, i, hA, :], exA[:, u, :],
                                             start=(i == 0), stop=(i == nst - 1))
                            nc.tensor.matmul(pvB, vt[:, i, hB, :], exB[:, u, :],
                                             start=(i == 0), stop=(i == nst - 1))
                    for pvx, p0 in ((pvA, 0), (pvB, 64)):
                        rz = nrmp.tile([1, 512], F32, tag="rz", name=f"rz{j}{hp}{p0}")
                        bz = nrmp.tile([64, 512], F32, tag="bz", name=f"bz{j}{hp}{p0}")
                        nc.vector.reciprocal(rz, pvx[DK:DK + 1, :])
                        nc.gpsimd.partition_broadcast(bz, rz)
                        nc.vector.tensor_mul(att_j[hp][p0:p0 + 64, :],
                                             pvx[0:DK, :], bz)
                for tt in range(4):
                    ot = outp.tile([128, D_MODEL], F32, tag="ot", name=f"ot{j}{tt}")
                    ttsl = slice(tt * 128, (tt + 1) * 128)
                    for eo in range(2):
                        pso = smps.tile([128, 512], F32, tag="sp", name=f"po{j}{tt}{eo}")
                        for f in range(4):
                            nc.tensor.matmul(pso, att_j[f][:, ttsl],
                                             wot[:, f, eo * 512:(eo + 1) * 512],
                                             start=(f == 0), stop=(f == 3))
                        nc.vector.tensor_copy(ot[:, eo * 512:(eo + 1) * 512], pso)
                    t0 = j * 512 + tt * 128
                    nc.sync.dma_start(out=out[t0:t0 + 128, :], in_=ot)


def build_nc():
    nc = bacc.Bacc(None, target_bir_lowering=False, debug=False)
    xT = nc.dram_tensor("xT", [D_MODEL, S], F32, kind="ExternalInput")
    wqT = nc.dram_tensor("wqT", [D_MODEL, E], F32, kind="ExternalInput")
    wkT = nc.dram_tensor("wkT", [D_MODEL, E], F32, kind="ExternalInput")
    wvT = nc.dram_tensor("wvT", [D_MODEL, E], F32, kind="ExternalInput")
    woT = nc.dram_tensor("woT", [E, D_MODEL], F32, kind="ExternalInput")
    out = nc.dram_tensor("out", [S, D_MODEL], F32, kind="ExternalOutput")
    with tile.TileContext(nc) as tc:
        _emit(tc, xT.ap(), wqT.ap(), wkT.ap(), wvT.ap(), woT.ap(), out.ap())
    nc.compile()
    return nc


def make_in_maps(x, W_Q, W_K, W_V, W_O):
    in_maps = []
    for c in range(8):
        b, g = divmod(c, 2)
        sl = slice(g * E, (g + 1) * E)
        in_maps.append({
            "xT": np.ascontiguousarray(x[b].T),
            "wqT": np.ascontiguousarray(W_Q[sl, :].T),
            "wkT": np.ascontiguousarray(W_K[sl, :].T),
            "wvT": np.ascontiguousarray(W_V[sl, :].T),
            "woT": np.ascontiguousarray(W_O[:, sl].T),
        })
    return in_maps


_NC_CACHE = None


def kernel(x, W_Q, W_K, W_V, W_O, _trace=False):
    global _NC_CACHE
    if _NC_CACHE is None:
        _NC_CACHE = build_nc()
    nc = _NC_CACHE
    in_maps = make_in_maps(x, W_Q, W_K, W_V, W_O)
    r = run_bass_kernel_spmd(nc, in_maps, list(range(8)), trace=_trace)
    kernel.last_result = r
    out = np.empty((B, S, D_MODEL), np.float32)
    for b in range(B):
        out[b] = r.results[2 * b]["out"] + r.results[2 * b + 1]["out"]
    return out


# revision 10
# speedup vs baseline: 488.2283x; 488.2283x over previous
"""Multi-head causal self-attention (B=4, S=2048, D=1024, H=16) on 8 trn2 cores.

Sharding: core c = (batch b = c//2, head-group g = c%2 of 8 heads).
Each core computes Q/K/V projections for its 8 heads over its batch's 2048
tokens, causal attention, and a PARTIAL output projection over its 512
feature dims. The host adds the two partial [2048, 1024] outputs per batch.
No on-device collectives.

On-core dataflow (all f32):
  QT[e,t], KT[e,t] = W @ x^T          (e on partitions -> head-dim-major)
  V'[t, h, 0:64] = x @ Wv^T, col 64 = 1.0   (ones column makes PV matmul
                                             also produce the softmax sum Z)
  St[s,q] = K^T-slices .T @ Q^T-slices      (scores transposed; K=64 row-
                                             packed pairs of heads)
  E = exp(St/8) via ACT from PSUM; causal mask via gpsimd.affine_select
  numerT[dv,q], Z[q] = V'^T.T @ E            (accumulated over s-tiles)
  attnT = numerT * (1/Z broadcast)
  out[t,e] = attnT-slices .T @ Wo^T-slices   (partial over this core's dims)
"""

import numpy as np

import concourse.bass as bass
import concourse.mybir as mybir
import concourse.tile as tile
from concourse import bacc
from concourse.bass_utils import run_bass_kernel_spmd

F32 = mybir.dt.float32
F32R = mybir.dt.float32r  # PE full-rate fp32 mode (N>=256)
MM_DT = F32R
AF = mybir.ActivationFunctionType

B = 4
S = 2048
D_MODEL = 1024
E = 512          # feature dims per core (8 heads x 64)
HEADS = 8        # heads per core
DK = 64
NQ = 4           # 512-token q-slices
ND = 8           # 128-dim d_model tiles
NT = 16          # 128-token tiles
SCALE = 0.125    # 1/sqrt(dk)


# phase switches for cost-model bisection (always all-True in production)
DO_QKV = True
DO_ATTN = True
DO_WO = True


def _emit(tc, xT, wqT, wkT, wvT, woT, out):
    nc = tc.nc
    with tc.tile_pool(name="singles", bufs=1) as singles:
        qt = [singles.tile([128, S], MM_DT, name=f"qt{i}") for i in range(4)]
        kt = [singles.tile([128, S], MM_DT, name=f"kt{i}") for i in range(4)]
        vt = singles.tile([128, NT, HEADS, DK + 1], MM_DT, name="vt")
        wot = singles.tile([128, 4, D_MODEL], MM_DT, name="wot")

        nc.sync.dma_start(out=wot, in_=woT.rearrange("(n p) m -> p n m", p=128))
        nc.vector.memset(vt[:, :, :, DK:DK + 1], 1.0)

        # ---------------- Phase 1: Q/K/V projections ----------------
        with (
            tc.tile_pool(name="wpool", bufs=1) as wpool,
            tc.tile_pool(name="xtc", bufs=9) as xtp,
            tc.tile_pool(name="qkps", bufs=4, space="PSUM") as qkps,
        ):
            wq = wpool.tile([128, ND, E], MM_DT, name="wq")
            wk = wpool.tile([128, ND, E], MM_DT, name="wk")
            wv = wpool.tile([128, ND, E], MM_DT, name="wv")
            nc.sync.dma_start(out=wq, in_=wqT.rearrange("(n p) m -> p n m", p=128))
            nc.sync.dma_start(out=wk, in_=wkT.rearrange("(n p) m -> p n m", p=128))
            nc.sync.dma_start(out=wv, in_=wvT.rearrange("(n p) m -> p n m", p=128))
            for j in range(NQ if DO_QKV else 0):
                tsl = slice(j * 512, (j + 1) * 512)
                chunks = []
                for d in range(ND):
                    ck = xtp.tile([128, 512], MM_DT, tag="xc", name=f"xc{j}_{d}")
                    nc.sync.dma_start(out=ck, in_=xT[d * 128:(d + 1) * 128, tsl])
                    chunks.append(ck)
                for w, dst in ((wq, qt), (wk, kt)):
                    for e in range(4):
                        esl = slice(e * 128, (e + 1) * 128)
                        ps = qkps.tile([128, 512], F32, tag="qk", name=f"ps{j}_{e}")
                        for d in range(ND):
                            nc.tensor.matmul(ps, w[:, d, esl], chunks[d],
                                             start=(d == 0), stop=(d == ND - 1))
                        nc.vector.tensor_copy(dst[e][:, tsl], ps)
                for u in range(4):
                    i = 4 * j + u
                    usl = slice(u * 128, (u + 1) * 128)
                    ps = qkps.tile([128, 512], F32, tag="qk", name=f"psv{j}_{u}")
                    for d in range(ND):
                        nc.tensor.matmul(ps, chunks[d][:, usl], wv[:, d, :],
                                         start=(d == 0), stop=(d == ND - 1))
                    nc.vector.tensor_copy(
                        vt[:, i, :, 0:DK],
                        ps.rearrange("p (h k) -> p h k", h=HEADS))

        # ------------- Phase 2+3: attention + output projection -------------
        with (
            tc.tile_pool(name="stps", bufs=3, space="PSUM") as stps,
            tc.tile_pool(name="smps", bufs=2, space="PSUM") as smps,
            tc.tile_pool(name="expp", bufs=6) as expp,
            tc.tile_pool(name="attp", bufs=8) as attp,
            tc.tile_pool(name="nrmp", bufs=2) as nrmp,
            tc.tile_pool(name="outp", bufs=2) as outp,
        ):
            for j in range(NQ if (DO_ATTN or DO_WO) else 0):
                qsl = slice(j * 512, (j + 1) * 512)
                att_j = [attp.tile([128, 512], MM_DT, tag="at", name=f"at{j}_{f}")
                         for f in range(4)]
                for hp in range(4 if DO_ATTN else 0):
                    hA, hB = 2 * hp, 2 * hp + 1
                    pvA = smps.tile([DK + 1, 512], F32, tag="sp", name=f"pvA{j}{hp}")
                    pvB = smps.tile([DK + 1, 512], F32, tag="sp", name=f"pvB{j}{hp}")
                    nst = 4 * (j + 1)
                    for g in range(nst // 2):
                        stA = stps.tile([128, 2, 512], F32, tag="st", name=f"stA{j}{hp}{g}")
                        stB = stps.tile([128, 2, 512], F32, tag="st", name=f"stB{j}{hp}{g}")
                        for u in range(2):
                            i = 2 * g + u
                            ssl = slice(i * 128, (i + 1) * 128)
                            nc.tensor.matmul(stA[:, u, :], kt[hp][0:64, ssl],
                                             qt[hp][0:64, qsl], start=True, stop=True)
                            nc.tensor.matmul(stB[:, u, :], kt[hp][64:128, ssl],
                                             qt[hp][64:128, qsl], start=True, stop=True)
                        exA = expp.tile([128, 2, 512], MM_DT, tag="ex", name=f"exA{j}{hp}{g}")
                        exB = expp.tile([128, 2, 512], MM_DT, tag="ex", name=f"exB{j}{hp}{g}")
                        nc.scalar.activation(exA, stA, AF.Exp, scale=SCALE)
                        nc.scalar.activation(exB, stB, AF.Exp, scale=SCALE)
                        for u in range(2):
                            i = 2 * g + u
                            r = i - 4 * j
                            if r >= 0:  # diagonal tile: zero where s > q
                                for ex in (exA, exB):
                                    nc.gpsimd.affine_select(
                                        out=ex[:, u, :], in_=ex[:, u, :],
                                        pattern=[[1, 512]],
                                        compare_op=mybir.AluOpType.is_ge,
                                        fill=0.0, base=-128 * r,
                                        channel_multiplier=-1)
                            nc.tensor.matmul(pvA, vt[:, i, hA, :], exA[:, u, :],
                                             start=(i == 0), stop=(i == nst - 1))
                            nc.tensor.matmul(pvB, vt[:, i, hB, :], exB[:, u, :],
                                             start=(i == 0), stop=(i == nst - 1))
                    for pvx, p0 in ((pvA, 0), (pvB, 64)):
                        rz = nrmp.tile([1, 512], F32, tag="rz", name=f"rz{j}{hp}{p0}")
                        bz = nrmp.tile([64, 512], F32, tag="bz", name=f"bz{j}{hp}{p0}")
                        nc.vector.reciprocal(rz, pvx[DK:DK + 1, :])
                        nc.gpsimd.partition_broadcast(bz, rz)
                        nc.vector.tensor_mul(att_j[hp][p0:p0 + 64, :],
                                             pvx[0:DK, :], bz)
                for tt in range(4 if DO_WO else 0):
                    ot = outp.tile([128, D_MODEL], F32, tag="ot", name=f"ot{j}{tt}")
                    ttsl = slice(tt * 128, (tt + 1) * 128)
                    for eo in range(2):
                        pso = smps.tile([128, 512], F32, tag="sp", name=f"po{j}{tt}{eo}")
                        for f in range(4):
                            nc.tensor.matmul(pso, att_j[f][:, ttsl],
                                             wot[:, f, eo * 512:(eo + 1) * 512],
                                             start=(f == 0), stop=(f == 3))
                        nc.vector.tensor_copy(ot[:, eo * 512:(eo + 1) * 512], pso)
                    t0 = j * 512 + tt * 128
                    nc.sync.dma_start(out=out[t0:t0 + 128, :], in_=ot)


def build_nc(reps=1):
    nc = bacc.Bacc(None, target_bir_lowering=False, debug=False)
    xT = nc.dram_tensor("xT", [D_MODEL, S], MM_DT, kind="ExternalInput")
    wqT = nc.dram_tensor("wqT", [D_MODEL, E], MM_DT, kind="ExternalInput")
    wkT = nc.dram_tensor("wkT", [D_MODEL, E], MM_DT, kind="ExternalInput")
    wvT = nc.dram_tensor("wvT", [D_MODEL, E], MM_DT, kind="ExternalInput")
    woT = nc.dram_tensor("woT", [E, D_MODEL], MM_DT, kind="ExternalInput")
    out = nc.dram_tensor("out", [S, D_MODEL], F32, kind="ExternalOutput")
    aps = (xT.ap(), wqT.ap(), wkT.ap(), wvT.ap(), woT.ap(), out.ap())
    with tile.TileContext(nc) as tc:
        if reps == 1:
            _emit(tc, *aps)
        else:
            with tc.For_i(0, reps, 1):
                _emit(tc, *aps)
    nc.compile()
    return nc


def make_in_maps(x, W_Q, W_K, W_V, W_O):
    in_maps = []
    for c in range(8):
        b, g = divmod(c, 2)
        sl = slice(g * E, (g + 1) * E)
        in_maps.append({
            "xT": np.ascontiguousarray(x[b].T),
            "wqT": np.ascontiguousarray(W_Q[sl, :].T),
            "wkT": np.ascontiguousarray(W_K[sl, :].T),
            "wvT": np.ascontiguousarray(W_V[sl, :].T),
            "woT": np.ascontiguousarray(W_O[:, sl].T),
        })
    return in_maps


_NC_CACHE = None


def kernel(x, W_Q, W_K, W_V, W_O, _trace=False):
    global _NC_CACHE
    if _NC_CACHE is None:
        _NC_CACHE = build_nc()
    nc = _NC_CACHE
    in_maps = make_in_maps(x, W_Q, W_K, W_V, W_O)
    r = run_bass_kernel_spmd(nc, in_maps, list(range(8)), trace=_trace)
    kernel.last_result = r
    out = np.empty((B, S, D_MODEL), np.float32)
    for b in range(B):
        out[b] = r.results[2 * b]["out"] + r.results[2 * b + 1]["out"]
    return out


# revision 16
# speedup vs baseline: 780.8266x; 1.5993x over previous
"""Multi-head causal self-attention (B=4, S=2048, D=1024, H=16) on 8 trn2 cores.

Sharding: core c = (batch b = c//2, head-group g = c%2 of 8 heads).
Each core computes Q/K/V projections for its 8 heads over its batch's 2048
tokens, causal attention, and a PARTIAL output projection over its 512
feature dims. The host adds the two partial [2048, 1024] outputs per batch.
No on-device collectives.

On-core dataflow (all f32):
  QT[e,t], KT[e,t] = W @ x^T          (e on partitions -> head-dim-major)
  V'[t, h, 0:64] = x @ Wv^T, col 64 = 1.0   (ones column makes PV matmul
                                             also produce the softmax sum Z)
  St[s,q] = K^T-slices .T @ Q^T-slices      (scores transposed; K=64 row-
                                             packed pairs of heads)
  E = exp(St/8) via ACT from PSUM; causal mask via gpsimd.affine_select
  numerT[dv,q], Z[q] = V'^T.T @ E            (accumulated over s-tiles)
  attnT = numerT * (1/Z broadcast)
  out[t,e] = attnT-slices .T @ Wo^T-slices   (partial over this core's dims)
"""

import numpy as np

import concourse.bass as bass
import concourse.mybir as mybir
import concourse.tile as tile
from concourse import bacc
from concourse.bass_utils import run_bass_kernel_spmd

F32 = mybir.dt.float32
F32R = mybir.dt.float32r  # PE full-rate fp32 mode (N>=256)
MM_DT = F32R
AF = mybir.ActivationFunctionType

B = 4
S = 2048
D_MODEL = 1024
E = 512          # feature dims per core (8 heads x 64)
HEADS = 8        # heads per core
DK = 64
NQ = 4           # 512-token q-slices
ND = 8           # 128-dim d_model tiles
NT = 16          # 128-token tiles
SCALE = 0.125    # 1/sqrt(dk)


# phase switches for cost-model bisection (always all-True in production)
DO_QKV = True
DO_ATTN = True
DO_WO = True


def _emit(tc, xT, wqT, wkT, wvT, woT, out):
    nc = tc.nc
    with tc.tile_pool(name="singles", bufs=1) as singles:
        qt = [singles.tile([128, S], MM_DT, name=f"qt{i}") for i in range(4)]
        kt = [singles.tile([128, S], MM_DT, name=f"kt{i}") for i in range(4)]
        vt = singles.tile([128, NT, HEADS, DK + 1], MM_DT, name="vt")
        wot = singles.tile([128, 4, D_MODEL], MM_DT, name="wot")

        ident = singles.tile([128, 128], MM_DT, name="ident")
        cmask = singles.tile([128, 4, 512], MM_DT, name="cmask")

        nc.sync.dma_start(out=wot, in_=woT)
        with tc.tile_pool(name="scratch", bufs=1) as scratch:
            sc = scratch.tile([128, 4, 512], F32, name="sc")
            nc.vector.memset(sc[:, 0, 0:128], 1.0)
            nc.vector.tensor_copy(vt[:, :, :, DK:DK + 1],
                                  sc[:, 0, 0:128].rearrange("p (a b) -> p a b", a=NT))
            # identity: keep 1.0 on the diagonal, 0 elsewhere
            nc.gpsimd.memset(sc[:, 0, 0:128], 0.0)
            nc.gpsimd.affine_select(
                out=sc[:, 0, 0:128], in_=sc[:, 0, 0:128], pattern=[[-1, 128]],
                compare_op=mybir.AluOpType.not_equal, fill=1.0,
                base=0, channel_multiplier=1)
            nc.vector.tensor_copy(ident, sc[:, 0, 0:128])
            # causal masks m_r[s, q] = 0 where s <= q - 128r else -1e9
            nc.gpsimd.memset(sc, 0.0)
            for r in range(4):
                nc.gpsimd.affine_select(
                    out=sc[:, r, :], in_=sc[:, r, :], pattern=[[1, 512]],
                    compare_op=mybir.AluOpType.is_ge, fill=-1e9,
                    base=-128 * r, channel_multiplier=-1)
            nc.vector.tensor_copy(cmask, sc)

        # ---------------- Phase 1: Q/K/V projections ----------------
        with (
            tc.tile_pool(name="wpool", bufs=1) as wpool,
            tc.tile_pool(name="xtc", bufs=9) as xtp,
            tc.tile_pool(name="qkps", bufs=4, space="PSUM") as qkps,
        ):
            wq = wpool.tile([128, ND, E], MM_DT, name="wq")
            wk = wpool.tile([128, ND, E], MM_DT, name="wk")
            wv = wpool.tile([128, ND, E], MM_DT, name="wv")
            nc.sync.dma_start(out=wq, in_=wqT)
            nc.sync.dma_start(out=wk, in_=wkT)
            nc.sync.dma_start(out=wv, in_=wvT)
            for j in range(NQ if DO_QKV else 0):
                tsl = slice(j * 512, (j + 1) * 512)
                chunks = []
                for d in range(ND):
                    ck = xtp.tile([128, 512], MM_DT, tag="xc", name=f"xc{j}_{d}")
                    nc.sync.dma_start(out=ck, in_=xT[j, d])
                    chunks.append(ck)
                for w, dst in ((wq, qt), (wk, kt)):
                    for e in range(4):
                        esl = slice(e * 128, (e + 1) * 128)
                        ps = qkps.tile([128, 512], F32, tag="qk", name=f"ps{j}_{e}")
                        for d in range(ND):
                            nc.tensor.matmul(ps, w[:, d, esl], chunks[d],
                                             start=(d == 0), stop=(d == ND - 1))
                        nc.vector.tensor_copy(dst[e][:, tsl], ps)
                for u in range(4):
                    i = 4 * j + u
                    usl = slice(u * 128, (u + 1) * 128)
                    ps = qkps.tile([128, 512], F32, tag="qk", name=f"psv{j}_{u}")
                    for d in range(ND):
                        nc.tensor.matmul(ps, chunks[d][:, usl], wv[:, d, :],
                                         start=(d == 0), stop=(d == ND - 1))
                    nc.vector.tensor_copy(
                        vt[:, i, :, 0:DK],
                        ps.rearrange("p (h k) -> p h k", h=HEADS))

        # ------------- Phase 2+3: attention + output projection -------------
        with (
            tc.tile_pool(name="stps", bufs=3, space="PSUM") as stps,
            tc.tile_pool(name="smps", bufs=2, space="PSUM") as smps,
            tc.tile_pool(name="expp", bufs=6) as expp,
            tc.tile_pool(name="attp", bufs=8) as attp,
            tc.tile_pool(name="nrmp", bufs=2) as nrmp,
            tc.tile_pool(name="outp", bufs=2) as outp,
        ):
            for j in range(NQ if (DO_ATTN or DO_WO) else 0):
                qsl = slice(j * 512, (j + 1) * 512)
                att_j = [attp.tile([128, 512], MM_DT, tag="at", name=f"at{j}_{f}")
                         for f in range(4)]
                for hp in range(4 if DO_ATTN else 0):
                    hA, hB = 2 * hp, 2 * hp + 1
                    pvA = smps.tile([DK + 1, 512], F32, tag="sp", name=f"pvA{j}{hp}")
                    pvB = smps.tile([DK + 1, 512], F32, tag="sp", name=f"pvB{j}{hp}")
                    nst = 4 * (j + 1)
                    for g in range(nst // 2):
                        stA = stps.tile([128, 2, 512], F32, tag="st", name=f"stA{j}{hp}{g}")
                        stB = stps.tile([128, 2, 512], F32, tag="st", name=f"stB{j}{hp}{g}")
                        for u in range(2):
                            i = 2 * g + u
                            r = i - 4 * j
                            ssl = slice(i * 128, (i + 1) * 128)
                            diag = r >= 0
                            # diag tiles: scores only over the valid q-range
                            # [128r, 512); the full-width mask matmul then
                            # overwrites the unwritten region (has_written=0)
                            # with -1e9 and adds exact 0.0 on the valid part.
                            q0 = 128 * r if diag else 0
                            qv = slice(j * 512 + q0, (j + 1) * 512)
                            if diag:
                                # mask first (full width, initializes bank),
                                # then scores accumulate over the valid range
                                nc.tensor.matmul(stA[:, u, :], ident,
                                                 cmask[:, r, :], start=True,
                                                 stop=False)
                                nc.tensor.matmul(stB[:, u, :], ident,
                                                 cmask[:, r, :], start=True,
                                                 stop=False)
                            nc.tensor.matmul(stA[:, u, q0:], kt[hp][0:64, ssl],
                                             qt[hp][0:64, qv], start=not diag,
                                             stop=True, tile_position=(0, 0))
                            nc.tensor.matmul(stB[:, u, q0:], kt[hp][64:128, ssl],
                                             qt[hp][64:128, qv], start=not diag,
                                             stop=True, tile_position=(64, 0))
                        exA = expp.tile([128, 2, 512], MM_DT, tag="ex", name=f"exA{j}{hp}{g}")
                        exB = expp.tile([128, 2, 512], MM_DT, tag="ex", name=f"exB{j}{hp}{g}")
                        nc.scalar.activation(exA, stA, AF.Exp, scale=SCALE)
                        nc.scalar.activation(exB, stB, AF.Exp, scale=SCALE)
                        for u in range(2):
                            i = 2 * g + u
                            nc.tensor.matmul(pvA, vt[:, i, hA, :], exA[:, u, :],
                                             start=(i == 0), stop=(i == nst - 1))
                            nc.tensor.matmul(pvB, vt[:, i, hB, :], exB[:, u, :],
                                             start=(i == 0), stop=(i == nst - 1))
                    for pvx, p0 in ((pvA, 0), (pvB, 64)):
                        rz = nrmp.tile([1, 512], F32, tag="rz", name=f"rz{j}{hp}{p0}")
                        bz = nrmp.tile([64, 512], F32, tag="bz", name=f"bz{j}{hp}{p0}")
                        nc.vector.reciprocal(rz, pvx[DK:DK + 1, :])
                        nc.gpsimd.partition_broadcast(bz, rz)
                        nc.vector.tensor_mul(att_j[hp][p0:p0 + 64, :],
                                             pvx[0:DK, :], bz)
                for tt in range(4 if DO_WO else 0):
                    ot = outp.tile([128, D_MODEL], F32, tag="ot", name=f"ot{j}{tt}")
                    ttsl = slice(tt * 128, (tt + 1) * 128)
                    for eo in range(2):
                        pso = smps.tile([128, 512], F32, tag="sp", name=f"po{j}{tt}{eo}")
                        for f in range(4):
                            nc.tensor.matmul(pso, att_j[f][:, ttsl],
                                             wot[:, f, eo * 512:(eo + 1) * 512],
                                             start=(f == 0), stop=(f == 3))
                        nc.vector.tensor_copy(ot[:, eo * 512:(eo + 1) * 512], pso)
                    t0 = j * 512 + tt * 128
                    nc.sync.dma_start(out=out[t0:t0 + 128, :], in_=ot)


def build_nc(reps=1):
    nc = bacc.Bacc(None, target_bir_lowering=False, debug=False)
    xT = nc.dram_tensor("xT", [NQ, ND, 128, 512], MM_DT, kind="ExternalInput")
    wqT = nc.dram_tensor("wqT", [128, ND, E], MM_DT, kind="ExternalInput")
    wkT = nc.dram_tensor("wkT", [128, ND, E], MM_DT, kind="ExternalInput")
    wvT = nc.dram_tensor("wvT", [128, ND, E], MM_DT, kind="ExternalInput")
    woT = nc.dram_tensor("woT", [128, 4, D_MODEL], MM_DT, kind="ExternalInput")
    out = nc.dram_tensor("out", [S, D_MODEL], F32, kind="ExternalOutput")
    aps = (xT.ap(), wqT.ap(), wkT.ap(), wvT.ap(), woT.ap(), out.ap())
    with tile.TileContext(nc) as tc:
        if reps == 1:
            _emit(tc, *aps)
        else:
            with tc.For_i(0, reps, 1):
                _emit(tc, *aps)
    nc.compile()
    return nc


def make_in_maps(x, W_Q, W_K, W_V, W_O):
    in_maps = []
    for c in range(8):
        b, g = divmod(c, 2)
        sl = slice(g * E, (g + 1) * E)
        xt = x[b].T  # [D, S]
        xt4 = np.ascontiguousarray(
            xt.reshape(8, 128, 4, 512).transpose(2, 0, 1, 3))  # [j, d, 128, 512]
        def wtile(w):  # [D, E] -> [128, 8, E]
            return np.ascontiguousarray(w.reshape(8, 128, -1).transpose(1, 0, 2))
        in_maps.append({
            "xT": xt4,
            "wqT": wtile(W_Q[sl, :].T),
            "wkT": wtile(W_K[sl, :].T),
            "wvT": wtile(W_V[sl, :].T),
            "woT": np.ascontiguousarray(
                W_O[:, sl].T.reshape(4, 128, 1024).transpose(1, 0, 2)),
        })
    return in_maps


_NC_CACHE = None


def kernel(x, W_Q, W_K, W_V, W_O, _trace=False):
    global _NC_CACHE
    if _NC_CACHE is None:
        _NC_CACHE = build_nc()
    nc = _NC_CACHE
    in_maps = make_in_maps(x, W_Q, W_K, W_V, W_O)
    r = run_bass_kernel_spmd(nc, in_maps, list(range(8)), trace=_trace)
    kernel.last_result = r
    out = np.empty((B, S, D_MODEL), np.float32)
    for b in range(B):
        out[b] = r.results[2 * b]["out"] + r.results[2 * b + 1]["out"]
    return out
